# revision 12
# baseline (speedup 1.0000x reference)
"""Multi-head attention (B=8, N=2048, C=320, H=5, D=64) on 8 Trainium2 cores.

Sharding: data-parallel over batch — core b computes attention for x[b].
Weights replicated, no collectives.

Per-core engine plan (v2):
  - Scores S^T[m,n] in fp16 with PARITY ROW-PAIRING: q^T/k^T are stored
    per-head with the 64 d-features duplicated on partitions 64..127, so
    even-mt and odd-mt score matmuls use different PE row groups and run
    concurrently (~2x effective scores throughput).
  - exp is split between the scalar engine (true Exp -> fp8, scaled by 1/4
    via bias=-ln4 for fp8 range) and the vector engine (Schraudolph
    piecewise-linear 2^z: one tensor_scalar whose rounded uint8 output IS
    the fp8e4m3 encoding). The denominator is built from the same P values,
    so the PWL/quantization error largely normalizes out.
  - AV uses fp8 DoubleRow: stationary [V_h|1] m-block pairs [128,2,65]
    against P^T pairs [128,2,512] — half the AV matmuls; ones column gives
    the softmax denominator in row 64 of the output.
  - q/k/v projections: x^T kept as fp8 c-planes [128,2,N] + fp16 c-tail
    [64,N]; q/k pages produced via fp8-DoubleRow + fp16 tail directly into
    the duplicated layout (stationary w^T carries each head twice).
  - proj for head h runs inside head h+1's attention: denominators ride row
    64 of the oT eviction, are DMA'd (partition-shifting) to a base-0 row,
    PE-transposed per-gt into a [128,16] column, reciprocal'd once; one
    scalar_tensor_tensor per (gt,h) chains normalized projection partials
    with the bias seeding the chain.
  - gpsimd does the bulk x fp32->fp8/fp16 casts (it cannot touch PSUM).
"""

import numpy as np

import concourse.bacc as bacc
import concourse.tile as tile
from concourse import mybir
from concourse.bass_utils import run_bass_kernel_spmd
from concourse.masks import make_identity

FP32 = mybir.dt.float32
FP16 = mybir.dt.float16
FP8 = mybir.dt.float8e4
U8 = mybir.dt.uint8
AF = mybir.ActivationFunctionType
ALU = mybir.AluOpType
DRM = mybir.MatmulPerfMode.DoubleRow

B = 8
C = 320
H = 5
D = 64
SCALE = D ** -0.5
LN4 = 1.3862943611198906
# PWL fp8e4m3 exp bits: bits = 8*(log2e*SCALE*s - 2 + 7 + delta)
PWL_S1 = 1.4426950408889634 * SCALE * 8.0
PWL_S2 = 8.0 * (7.0 - 2.0 - 0.04295)
VP = 66  # v page stride per head (64 d + ones col + 1 pad)
VSTRIDE = 352  # per-mt stride in v_sb free dim (5*66=330 padded to 22*16)


def dve_exp_tile(h, nci, mt):
    """Which exp tiles go to the vector engine (PWL) vs scalar (true exp)."""
    return mt % 8 in (1, 3, 5)


def build_program(N: int):
    nc = bacc.Bacc("TRN2", target_bir_lowering=False, debug=False)

    x_d = nc.dram_tensor("x", [N, C], FP32, kind="ExternalInput")
    wqkv_d = nc.dram_tensor("w_qkv", [3 * C, C], FP32, kind="ExternalInput")
    wproj_d = nc.dram_tensor("w_proj", [C, C], FP32, kind="ExternalInput")
    bproj_d = nc.dram_tensor("b_proj", [C], FP32, kind="ExternalInput")
    out_d = nc.dram_tensor("out", [N, C], FP32, kind="ExternalOutput")

    MT = N // 128
    CHUNK = 1024
    assert N % CHUNK == 0 and MT % 2 == 0

    with tile.TileContext(nc) as tc:
        with (
            tc.tile_pool(name="per", bufs=1) as per,
            tc.tile_pool(name="ld", bufs=3) as ld,
            tc.tile_pool(name="s_ps", bufs=3, space="PSUM") as s_ps,
            tc.tile_pool(name="o_ps", bufs=1, space="PSUM") as o_ps,
            tc.tile_pool(name="pt", bufs=3) as pt_pool,
            tc.tile_pool(name="st", bufs=4) as st_pool,
        ):
            identity = per.tile([128, 128], FP32)
            make_identity(nc, identity[:])
            id16 = per.tile([128, 128], FP16)
            nc.vector.tensor_copy(id16[:], identity[:])

            # weights: transposed + head-duplicated layouts
            wplane = per.tile([128, 2, 2, 3, 128], FP16)  # [c, ci, qk, plane, col]
            wptl = per.tile([64, 2, 3, 128], FP16)  # c-tail 256..319
            wv = per.tile([128, 2, C], FP16)
            wv16 = per.tile([64, C], FP16)
            wpt = per.tile([64, H, C], FP16)
            bias_sb = per.tile([128, C], FP32)
            b_row = per.tile([1, C], FP32)
            ones1 = per.tile([1, 128], FP32)
            nc.gpsimd.memset(ones1[:], 1.0)
            bconst = per.tile([128, 1], FP32)
            nc.gpsimd.memset(bconst[:], -LN4)

            xTa = per.tile([128, 2, N], FP16)
            xT16 = per.tile([64, N], FP16)
            qTd = per.tile([128, H, N], FP16)
            kTd = per.tile([128, H, N], FP16)
            v_sb = per.tile([128, MT, VSTRIDE], FP16)
            oT = per.tile([65, H, N], FP16)  # rows 0..63 O^T/4, row 64 denom/4
            dstage = per.tile([1, H, N], FP16)
            recipT = per.tile([128, H, MT], FP32)
            acc_a = per.tile([128, MT, C], FP32)
            acc_b = per.tile([128, MT, C], FP32)

            v_heads = v_sb[:, :, 0 : H * VP].rearrange("p m (h e) -> p m h e", h=H)
            nc.gpsimd.memset(v_heads[:, :, :, D : D + 1], 1.0)

            # ---------------- weight prep ----------------
            # q/k rows of w_qkv: per qk, 3 blocks (128/128/64 rows)
            for qk in range(2):
                for wb in range(3):
                    r0 = qk * C + wb * 128
                    rp = min(128, C - wb * 128)
                    wnat = ld.tile([128, C], FP32, tag="wnat")
                    nc.sync.dma_start(wnat[:rp, :], wqkv_d.ap()[r0 : r0 + rp, :])
                    wnat_h = ld.tile([128, C], FP16, tag="wnat_h")
                    nc.vector.tensor_copy(wnat_h[:rp, :], wnat[:rp, :])
                    for ci in range(3):
                        c0, cp = ci * 128, (64 if ci == 2 else 128)
                        ps = s_ps.tile([128, 1024], FP32, tag="s")
                        psh = ps[:].bitcast(FP16)
                        nc.tensor.transpose(
                            psh[:cp, 0:rp], wnat_h[:rp, c0 : c0 + cp], id16[:rp, :rp]
                        )
                        if ci < 2:
                            nc.vector.tensor_copy(
                                wplane[:, ci, qk, wb, 0:rp], psh[:cp, 0:rp]
                            )
                        else:
                            nc.vector.tensor_copy(wptl[:, qk, wb, 0:rp], psh[:cp, 0:rp])
            # v rows of w_qkv
            for wb in range(3):
                r0 = 2 * C + wb * 128
                rp = min(128, C - wb * 128)
                wnat = ld.tile([128, C], FP32, tag="wnat")
                nc.sync.dma_start(wnat[:rp, :], wqkv_d.ap()[r0 : r0 + rp, :])
                wnat_h = ld.tile([128, C], FP16, tag="wnat_h")
                nc.vector.tensor_copy(wnat_h[:rp, :], wnat[:rp, :])
                for ci in range(3):
                    c0, cp = ci * 128, (64 if ci == 2 else 128)
                    ps = s_ps.tile([128, 1024], FP32, tag="s")
                    psh = ps[:].bitcast(FP16)
                    nc.tensor.transpose(
                        psh[:cp, 0:rp], wnat_h[:rp, c0 : c0 + cp], id16[:rp, :rp]
                    )
                    dst_cols = slice(wb * 128, wb * 128 + rp)
                    if ci < 2:
                        nc.vector.tensor_copy(wv[:, ci, dst_cols], psh[:cp, :rp])
                    else:
                        nc.vector.tensor_copy(wv16[:, dst_cols], psh[:cp, :rp])
            # w_proj -> wpt (emitted as jobs)
            def emit_wproj(wt):
                r0 = wt * 128
                rp = min(128, C - r0)
                wpnat = ld.tile([128, C], FP32, tag="wnat")
                nc.sync.dma_start(wpnat[:rp, :], wproj_d.ap()[r0 : r0 + rp, :])
                wpnat_h = ld.tile([128, C], FP16, tag="wnat_h")
                nc.vector.tensor_copy(wpnat_h[:rp, :], wpnat[:rp, :])
                for h in range(H):
                    ps = s_ps.tile([128, 1024], FP32, tag="s")
                    psh = ps[:].bitcast(FP16)
                    nc.tensor.transpose(
                        psh[:64, 0:rp],
                        wpnat_h[:rp, h * D : (h + 1) * D],
                        id16[:rp, :rp],
                    )
                    nc.vector.tensor_copy(wpt[:, h, r0 : r0 + rp], psh[:64, :rp])

            def emit_bias():
                nc.sync.dma_start(
                    b_row[:], bproj_d.ap().rearrange("(a c) -> a c", a=1)
                )
                ps = s_ps.tile([128, 1024], FP32, tag="s")
                nc.tensor.matmul(ps[:, :C], ones1[:], b_row[:], start=True, stop=True)
                nc.vector.tensor_copy(bias_sb[:], ps[:, :C])

            # ---------------- x load, cast, transpose ----------------
            x_re = x_d.ap().rearrange("(t p) c -> p t c", p=128)

            def emit_xgrp(g):
                gn = min(2, MT - g)
                xnat = ld.tile([128, 2, C], FP32, tag="xnat")
                nc.sync.dma_start(xnat[:, :gn, :], x_re[:, g : g + gn, :])
                xnath = ld.tile([128, 2, C], FP16, tag="xnath")
                eng = nc.vector if g <= 2 else nc.gpsimd
                eng.tensor_copy(xnath[:, :gn, :], xnat[:, :gn, :])
                for t in range(gn):
                    mt = g + t
                    ps = s_ps.tile([128, 1024], FP32, tag="s")
                    psh = ps[:].bitcast(FP16)
                    for ci in range(2):
                        nc.tensor.transpose(
                            psh[:, ci * 128 : ci * 128 + 128],
                            xnath[:, t, ci * 128 : ci * 128 + 128],
                            id16[:, :],
                        )
                    nc.tensor.transpose(
                        psh[:64, 256:384], xnath[:, t, 256:320], id16[:, :]
                    )
                    nc.vector.tensor_copy(
                        xTa[:, :, mt * 128 : (mt + 1) * 128],
                        psh[:, 0:256].rearrange("p (ci k) -> p ci k", ci=2),
                    )
                    nc.vector.tensor_copy(
                        xT16[:, mt * 128 : (mt + 1) * 128], psh[:64, 256:384]
                    )

            for g in range(0, MT, 2):
                emit_xgrp(g)

            # ---------------- producers ----------------
            def emit_qk_chunk(j, qk, s0):
                # plane j covers heads (2j, 2j+1); j=2 covers head 4 only
                dst = qTd if qk == 0 else kTd
                nh = 2 if j < 2 else 1
                rows = 64 * nh
                h0 = 2 * j
                ps = s_ps.tile([128, 1024], FP32, tag="s")
                for ci in range(2):
                    nc.tensor.matmul(
                        ps[:rows, 0:512],
                        wplane[:, ci, qk, j, 0:rows],
                        xTa[:, ci, s0 : s0 + 512],
                        start=(ci == 0),
                        stop=False,
                    )
                nc.tensor.matmul(
                    ps[:rows, 0:512],
                    wptl[:, qk, j, 0:rows],
                    xT16[:, s0 : s0 + 512],
                    start=False,
                    stop=True,
                )
                # lane-aligned evictions into page halves, then DMA-duplicate
                nc.vector.tensor_copy(dst[0:64, h0, s0 : s0 + 512], ps[0:64, 0:512])
                nc.sync.dma_start(
                    dst[64:128, h0, s0 : s0 + 512], dst[0:64, h0, s0 : s0 + 512]
                )
                if nh == 2:
                    nc.vector.tensor_copy(
                        dst[64:128, h0 + 1, s0 : s0 + 512], ps[64:128, 0:512]
                    )
                    nc.sync.dma_start(
                        dst[0:64, h0 + 1, s0 : s0 + 512],
                        dst[64:128, h0 + 1, s0 : s0 + 512],
                    )

            def qk_jobs(j):
                return [
                    (lambda q=qk, s=s0: emit_qk_chunk(j, q, s))
                    for qk in range(2)
                    for s0 in range(0, N, 512)
                ]

            def emit_v_tile(mt):
                ps = s_ps.tile([128, 1024], FP32, tag="s")
                for ci in range(2):
                    nc.tensor.matmul(
                        ps[:, :C],
                        xTa[:, ci, mt * 128 : (mt + 1) * 128],
                        wv[:, ci, :],
                        start=(ci == 0),
                        stop=False,
                    )
                nc.tensor.matmul(
                    ps[:, :C],
                    xT16[:, mt * 128 : (mt + 1) * 128],
                    wv16[:, :],
                    start=False,
                    stop=True,
                )
                nc.vector.tensor_copy(
                    v_heads[:, mt, :, 0:D],
                    ps[:, :C].rearrange("p (h e) -> p h e", h=H),
                )

            # ---------------- per-head recipT / proj ----------------
            def emit_recipT(h):
                rt0 = s_ps.tile([128, 1024], FP32, tag="s")
                rt = rt0[:, 0:32].bitcast(FP16)
                rtv = rt.rearrange("p (g two) -> p g two", two=2)
                for gt in range(MT):
                    nc.tensor.transpose(
                        rtv[:, gt, 0:1],
                        dstage[0:1, h, gt * 128 : (gt + 1) * 128],
                        id16[0:1, 0:1],
                    )
                nc.vector.tensor_copy(recipT[:, h, :], rtv[:, :MT, 0])
                nc.vector.reciprocal(recipT[:, h, :], recipT[:, h, :])

            def emit_proj(h, gt):
                yp = s_ps.tile([128, 1024], FP32, tag="s")
                nc.tensor.matmul(
                    yp[:, :C],
                    oT[0:64, h, gt * 128 : (gt + 1) * 128],
                    wpt[:, h, :],
                    start=True,
                    stop=True,
                )
                src = acc_a if h % 2 == 1 else acc_b
                dst = acc_a if h % 2 == 0 else acc_b
                prev = bias_sb[:] if h == 0 else src[:, gt, :]
                if h == H - 1:
                    stg = st_pool.tile([128, C], FP32, tag="ost")
                    nc.vector.scalar_tensor_tensor(
                        stg[:], yp[:, :C], recipT[:, h, gt : gt + 1], prev,
                        ALU.mult, ALU.add,
                    )
                    nc.sync.dma_start(out_d.ap()[gt * 128 : (gt + 1) * 128, :], stg[:])
                else:
                    nc.vector.scalar_tensor_tensor(
                        dst[:, gt, :], yp[:, :C], recipT[:, h, gt : gt + 1], prev,
                        ALU.mult, ALU.add,
                    )

            # ---------------- attention ----------------
            def emit_attention(h, nci, jobs):
                n0 = nci * CHUNK
                ot = o_ps.tile([65, CHUNK], FP32, tag="ot")

                def emit_av(mp, pt):
                    for mt in (mp, mp + 1):
                        for c in range(2):
                            nc.tensor.matmul(
                                ot[:, c * 512 : c * 512 + 512],
                                v_sb[:, mt, h * VP : h * VP + 65],
                                pt[:, mt % 2, c * 512 : c * 512 + 512],
                                start=(mt == 0),
                                stop=(mt == MT - 1),
                            )

                prev = None
                for mp in range(0, MT, 2):
                    if jobs:
                        jobs.pop(0)()
                    sp_e = s_ps.tile([128, 1024], FP32, tag="s")
                    sp_o = s_ps.tile([128, 1024], FP32, tag="s")
                    # interleave parities so the two PE row groups overlap
                    for c in range(2):
                        for par, sp in ((0, sp_e), (1, sp_o)):
                            rb = 64 * par
                            mt = mp + par
                            nc.tensor.matmul(
                                sp[:, c * 512 : c * 512 + 512],
                                kTd[rb : rb + 64, h, mt * 128 : (mt + 1) * 128],
                                qTd[rb : rb + 64, h, n0 + c * 512 : n0 + c * 512 + 512],
                                start=True,
                                stop=True,
                            )
                    if prev is not None:
                        emit_av(*prev)
                    pt = pt_pool.tile([128, 2, CHUNK], FP16, tag="pt")
                    nc.scalar.activation(
                        pt[:, 0, :], sp_e[:], AF.Exp, bias=bconst[:], scale=SCALE
                    )
                    nc.scalar.activation(
                        pt[:, 1, :], sp_o[:], AF.Exp, bias=bconst[:], scale=SCALE
                    )
                    if jobs:
                        jobs.pop(0)()
                    prev = (mp, pt)
                emit_av(*prev)
                for job in jobs:
                    job()
                nc.vector.tensor_copy(oT[:, h, n0 : n0 + CHUNK], ot[:])
                nc.sync.dma_start(
                    dstage[0:1, h, n0 : n0 + CHUNK], oT[64:65, h, n0 : n0 + CHUNK]
                )

            # ---------------- schedule ----------------
            emit_wproj(0)
            emit_wproj(1)
            emit_wproj(2)
            emit_bias()
            for s0 in range(0, N, 512):
                emit_qk_chunk(0, 0, s0)
                emit_qk_chunk(0, 1, s0)
            pending = [(lambda m=mt: emit_v_tile(m)) for mt in range(MT)]
            for h in range(H):
                nxt = qk_jobs(h + 1) if h + 1 <= 2 else []
                projh = []
                if h >= 1:
                    projh = [lambda hh=h - 1: emit_recipT(hh)] + [
                        (lambda hh=h - 1, g=gt: emit_proj(hh, g)) for gt in range(MT)
                    ]
                emit_attention(h, 0, pending + nxt[:4] + projh[:12])
                pending = []
                emit_attention(h, 1, nxt[4:] + projh[12:])
            emit_recipT(H - 1)
            for gt in range(MT):
                emit_proj(H - 1, gt)

    nc.compile()
    return nc


_cache = {}


def _get_program(N: int):
    if N not in _cache:
        _cache[N] = build_program(N)
    return _cache[N]


def kernel(x, w_qkv, w_proj, b_proj):
    x = np.ascontiguousarray(np.asarray(x, dtype=np.float32))
    w_qkv = np.ascontiguousarray(np.asarray(w_qkv, dtype=np.float32))
    w_proj = np.ascontiguousarray(np.asarray(w_proj, dtype=np.float32))
    b_proj = np.ascontiguousarray(np.asarray(b_proj, dtype=np.float32))
    Bx, N, Cx = x.shape
    assert Bx == B and Cx == C, (x.shape,)

    nc = _get_program(N)
    in_maps = [
        {"x": x[b], "w_qkv": w_qkv, "w_proj": w_proj, "b_proj": b_proj}
        for b in range(B)
    ]
    res = run_bass_kernel_spmd(nc, in_maps, core_ids=list(range(B)))
    return np.stack([res.results[b]["out"] for b in range(B)], axis=0)


# revision 13
# speedup vs baseline: 1.0268x; 1.0268x over previous
"""Multi-head attention (B=8, N=2048, C=320, H=5, D=64) on 8 Trainium2 cores.

Sharding: data-parallel over batch — core b computes attention for x[b].
Weights replicated, no collectives.

Per-core engine plan (v2):
  - Scores S^T[m,n] in fp16 with PARITY ROW-PAIRING: q^T/k^T are stored
    per-head with the 64 d-features duplicated on partitions 64..127, so
    even-mt and odd-mt score matmuls use different PE row groups and run
    concurrently (~2x effective scores throughput).
  - exp is split between the scalar engine (true Exp -> fp8, scaled by 1/4
    via bias=-ln4 for fp8 range) and the vector engine (Schraudolph
    piecewise-linear 2^z: one tensor_scalar whose rounded uint8 output IS
    the fp8e4m3 encoding). The denominator is built from the same P values,
    so the PWL/quantization error largely normalizes out.
  - AV uses fp8 DoubleRow: stationary [V_h|1] m-block pairs [128,2,65]
    against P^T pairs [128,2,512] — half the AV matmuls; ones column gives
    the softmax denominator in row 64 of the output.
  - q/k/v projections: x^T kept as fp8 c-planes [128,2,N] + fp16 c-tail
    [64,N]; q/k pages produced via fp8-DoubleRow + fp16 tail directly into
    the duplicated layout (stationary w^T carries each head twice).
  - proj for head h runs inside head h+1's attention: denominators ride row
    64 of the oT eviction, are DMA'd (partition-shifting) to a base-0 row,
    PE-transposed per-gt into a [128,16] column, reciprocal'd once; one
    scalar_tensor_tensor per (gt,h) chains normalized projection partials
    with the bias seeding the chain.
  - gpsimd does the bulk x fp32->fp8/fp16 casts (it cannot touch PSUM).
"""

import numpy as np

import concourse.bacc as bacc
import concourse.tile as tile
from concourse import mybir
from concourse.bass_utils import run_bass_kernel_spmd
from concourse.masks import make_identity

FP32 = mybir.dt.float32
FP16 = mybir.dt.float16
FP8 = mybir.dt.float8e4
U8 = mybir.dt.uint8
AF = mybir.ActivationFunctionType
ALU = mybir.AluOpType
DRM = mybir.MatmulPerfMode.DoubleRow

B = 8
C = 320
H = 5
D = 64
SCALE = D ** -0.5
LN4 = 1.3862943611198906
# PWL fp8e4m3 exp bits: bits = 8*(log2e*SCALE*s - 2 + 7 + delta)
PWL_S1 = 1.4426950408889634 * SCALE * 8.0
PWL_S2 = 8.0 * (7.0 - 2.0 - 0.04295)
VP = 66  # v page stride per head (64 d + ones col + 1 pad)
VSTRIDE = 352  # per-mt stride in v_sb free dim (5*66=330 padded to 22*16)


def dve_exp_tile(h, nci, mt):
    """Which exp tiles go to the vector engine (PWL) vs scalar (true exp)."""
    return mt % 8 in (1, 3, 5)


def build_program(N: int):
    nc = bacc.Bacc("TRN2", target_bir_lowering=False, debug=False)

    x_d = nc.dram_tensor("x", [N, C], FP32, kind="ExternalInput")
    wqkv_d = nc.dram_tensor("w_qkv", [3 * C, C], FP32, kind="ExternalInput")
    wproj_d = nc.dram_tensor("w_proj", [C, C], FP32, kind="ExternalInput")
    bproj_d = nc.dram_tensor("b_proj", [C], FP32, kind="ExternalInput")
    out_d = nc.dram_tensor("out", [N, C], FP32, kind="ExternalOutput")

    MT = N // 128
    CHUNK = 1024
    assert N % CHUNK == 0 and MT % 2 == 0

    with tile.TileContext(nc) as tc:
        with (
            tc.tile_pool(name="per", bufs=1) as per,
            tc.tile_pool(name="ld", bufs=3) as ld,
            tc.tile_pool(name="s_ps", bufs=3, space="PSUM") as s_ps,
            tc.tile_pool(name="o_ps", bufs=1, space="PSUM") as o_ps,
            tc.tile_pool(name="pt", bufs=3) as pt_pool,
            tc.tile_pool(name="st", bufs=4) as st_pool,
        ):
            identity = per.tile([128, 128], FP32)
            make_identity(nc, identity[:])
            id16 = per.tile([128, 128], FP16)
            nc.vector.tensor_copy(id16[:], identity[:])

            # weights: transposed + head-duplicated layouts
            wplane = per.tile([128, 2, 2, 3, 128], FP16)  # [c, ci, qk, plane, col]
            wptl = per.tile([64, 2, 3, 128], FP16)  # c-tail 256..319
            wv = per.tile([128, 2, C], FP16)
            wv16 = per.tile([64, C], FP16)
            wpt = per.tile([64, H, C], FP16)
            bias_sb = per.tile([128, C], FP32)
            b_row = per.tile([1, C], FP32)
            ones1 = per.tile([1, 128], FP32)
            nc.gpsimd.memset(ones1[:], 1.0)
            bconst = per.tile([128, 1], FP32)
            nc.gpsimd.memset(bconst[:], -LN4)

            xTa = per.tile([128, 2, N], FP16)
            xT16 = per.tile([64, N], FP16)
            qTd = per.tile([128, H, N], FP16)
            kTd = per.tile([128, H, N], FP16)
            v_sb = per.tile([128, MT, VSTRIDE], FP16)
            oT = per.tile([65, H, N], FP16)  # rows 0..63 O^T/4, row 64 denom/4
            dstage = per.tile([1, H, N], FP16)
            recipT = per.tile([128, H, MT], FP32)
            acc_a = per.tile([128, MT, C], FP32)
            acc_b = per.tile([128, MT, C], FP32)

            v_heads = v_sb[:, :, 0 : H * VP].rearrange("p m (h e) -> p m h e", h=H)
            nc.gpsimd.memset(v_heads[:, :, :, D : D + 1], 1.0)

            # ---------------- weight prep ----------------
            # q/k rows of w_qkv: per qk, 3 blocks (128/128/64 rows)
            for qk in range(2):
                for wb in range(3):
                    r0 = qk * C + wb * 128
                    rp = min(128, C - wb * 128)
                    wnat = ld.tile([128, C], FP32, tag="wnat")
                    nc.sync.dma_start(wnat[:rp, :], wqkv_d.ap()[r0 : r0 + rp, :])
                    wnat_h = ld.tile([128, C], FP16, tag="wnat_h")
                    nc.vector.tensor_copy(wnat_h[:rp, :], wnat[:rp, :])
                    for ci in range(3):
                        c0, cp = ci * 128, (64 if ci == 2 else 128)
                        ps = s_ps.tile([128, 1024], FP32, tag="s")
                        psh = ps[:].bitcast(FP16)
                        nc.tensor.transpose(
                            psh[:cp, 0:rp], wnat_h[:rp, c0 : c0 + cp], id16[:rp, :rp]
                        )
                        if ci < 2:
                            nc.vector.tensor_copy(
                                wplane[:, ci, qk, wb, 0:rp], psh[:cp, 0:rp]
                            )
                        else:
                            nc.vector.tensor_copy(wptl[:, qk, wb, 0:rp], psh[:cp, 0:rp])
            # v rows of w_qkv
            for wb in range(3):
                r0 = 2 * C + wb * 128
                rp = min(128, C - wb * 128)
                wnat = ld.tile([128, C], FP32, tag="wnat")
                nc.sync.dma_start(wnat[:rp, :], wqkv_d.ap()[r0 : r0 + rp, :])
                wnat_h = ld.tile([128, C], FP16, tag="wnat_h")
                nc.vector.tensor_copy(wnat_h[:rp, :], wnat[:rp, :])
                for ci in range(3):
                    c0, cp = ci * 128, (64 if ci == 2 else 128)
                    ps = s_ps.tile([128, 1024], FP32, tag="s")
                    psh = ps[:].bitcast(FP16)
                    nc.tensor.transpose(
                        psh[:cp, 0:rp], wnat_h[:rp, c0 : c0 + cp], id16[:rp, :rp]
                    )
                    dst_cols = slice(wb * 128, wb * 128 + rp)
                    if ci < 2:
                        nc.vector.tensor_copy(wv[:, ci, dst_cols], psh[:cp, :rp])
                    else:
                        nc.vector.tensor_copy(wv16[:, dst_cols], psh[:cp, :rp])
            # w_proj -> wpt (emitted as jobs)
            def emit_wproj(wt):
                r0 = wt * 128
                rp = min(128, C - r0)
                wpnat = ld.tile([128, C], FP32, tag="wnat")
                nc.sync.dma_start(wpnat[:rp, :], wproj_d.ap()[r0 : r0 + rp, :])
                wpnat_h = ld.tile([128, C], FP16, tag="wnat_h")
                nc.vector.tensor_copy(wpnat_h[:rp, :], wpnat[:rp, :])
                for h in range(H):
                    ps = s_ps.tile([128, 1024], FP32, tag="s")
                    psh = ps[:].bitcast(FP16)
                    nc.tensor.transpose(
                        psh[:64, 0:rp],
                        wpnat_h[:rp, h * D : (h + 1) * D],
                        id16[:rp, :rp],
                    )
                    nc.vector.tensor_copy(wpt[:, h, r0 : r0 + rp], psh[:64, :rp])

            def emit_bias():
                nc.sync.dma_start(
                    b_row[:], bproj_d.ap().rearrange("(a c) -> a c", a=1)
                )
                ps = s_ps.tile([128, 1024], FP32, tag="s")
                nc.tensor.matmul(ps[:, :C], ones1[:], b_row[:], start=True, stop=True)
                nc.vector.tensor_copy(bias_sb[:], ps[:, :C])

            # ---------------- x load, cast, transpose ----------------
            x_re = x_d.ap().rearrange("(t p) c -> p t c", p=128)

            def emit_xgrp(g):
                gn = min(2, MT - g)
                xnat = ld.tile([128, 2, C], FP32, tag="xnat")
                nc.sync.dma_start(xnat[:, :gn, :], x_re[:, g : g + gn, :])
                xnath = ld.tile([128, 2, C], FP16, tag="xnath")
                eng = nc.vector if g <= 6 else nc.gpsimd
                eng.tensor_copy(xnath[:, :gn, :], xnat[:, :gn, :])
                for t in range(gn):
                    mt = g + t
                    ps = s_ps.tile([128, 1024], FP32, tag="s")
                    psh = ps[:].bitcast(FP16)
                    for ci in range(2):
                        nc.tensor.transpose(
                            psh[:, ci * 128 : ci * 128 + 128],
                            xnath[:, t, ci * 128 : ci * 128 + 128],
                            id16[:, :],
                        )
                    nc.tensor.transpose(
                        psh[:64, 256:384], xnath[:, t, 256:320], id16[:, :]
                    )
                    nc.vector.tensor_copy(
                        xTa[:, :, mt * 128 : (mt + 1) * 128],
                        psh[:, 0:256].rearrange("p (ci k) -> p ci k", ci=2),
                    )
                    nc.vector.tensor_copy(
                        xT16[:, mt * 128 : (mt + 1) * 128], psh[:64, 256:384]
                    )

            for g in (0, 2, 4, 6):
                emit_xgrp(g)

            # ---------------- producers ----------------
            def emit_qk_chunk(j, qk, s0):
                # plane j covers heads (2j, 2j+1); j=2 covers head 4 only
                dst = qTd if qk == 0 else kTd
                nh = 2 if j < 2 else 1
                rows = 64 * nh
                h0 = 2 * j
                ps = s_ps.tile([128, 1024], FP32, tag="s")
                for ci in range(2):
                    nc.tensor.matmul(
                        ps[:rows, 0:512],
                        wplane[:, ci, qk, j, 0:rows],
                        xTa[:, ci, s0 : s0 + 512],
                        start=(ci == 0),
                        stop=False,
                    )
                nc.tensor.matmul(
                    ps[:rows, 0:512],
                    wptl[:, qk, j, 0:rows],
                    xT16[:, s0 : s0 + 512],
                    start=False,
                    stop=True,
                )
                # lane-aligned evictions into page halves, then DMA-duplicate
                nc.vector.tensor_copy(dst[0:64, h0, s0 : s0 + 512], ps[0:64, 0:512])
                nc.sync.dma_start(
                    dst[64:128, h0, s0 : s0 + 512], dst[0:64, h0, s0 : s0 + 512]
                )
                if nh == 2:
                    nc.vector.tensor_copy(
                        dst[64:128, h0 + 1, s0 : s0 + 512], ps[64:128, 0:512]
                    )
                    nc.sync.dma_start(
                        dst[0:64, h0 + 1, s0 : s0 + 512],
                        dst[64:128, h0 + 1, s0 : s0 + 512],
                    )

            def qk_jobs(j):
                return [
                    (lambda q=qk, s=s0: emit_qk_chunk(j, q, s))
                    for qk in range(2)
                    for s0 in range(0, N, 512)
                ]

            def emit_v_tile(mt):
                ps = s_ps.tile([128, 1024], FP32, tag="s")
                for ci in range(2):
                    nc.tensor.matmul(
                        ps[:, :C],
                        xTa[:, ci, mt * 128 : (mt + 1) * 128],
                        wv[:, ci, :],
                        start=(ci == 0),
                        stop=False,
                    )
                nc.tensor.matmul(
                    ps[:, :C],
                    xT16[:, mt * 128 : (mt + 1) * 128],
                    wv16[:, :],
                    start=False,
                    stop=True,
                )
                nc.vector.tensor_copy(
                    v_heads[:, mt, :, 0:D],
                    ps[:, :C].rearrange("p (h e) -> p h e", h=H),
                )

            # ---------------- per-head recipT / proj ----------------
            def emit_recipT(h):
                rt0 = s_ps.tile([128, 1024], FP32, tag="s")
                rt = rt0[:, 0:32].bitcast(FP16)
                rtv = rt.rearrange("p (g two) -> p g two", two=2)
                for gt in range(MT):
                    nc.tensor.transpose(
                        rtv[:, gt, 0:1],
                        dstage[0:1, h, gt * 128 : (gt + 1) * 128],
                        id16[0:1, 0:1],
                    )
                nc.vector.tensor_copy(recipT[:, h, :], rtv[:, :MT, 0])
                nc.vector.reciprocal(recipT[:, h, :], recipT[:, h, :])

            def emit_proj(h, gt):
                yp = s_ps.tile([128, 1024], FP32, tag="s")
                nc.tensor.matmul(
                    yp[:, :C],
                    oT[0:64, h, gt * 128 : (gt + 1) * 128],
                    wpt[:, h, :],
                    start=True,
                    stop=True,
                )
                src = acc_a if h % 2 == 1 else acc_b
                dst = acc_a if h % 2 == 0 else acc_b
                prev = bias_sb[:] if h == 0 else src[:, gt, :]
                if h == H - 1:
                    stg = st_pool.tile([128, C], FP32, tag="ost")
                    nc.vector.scalar_tensor_tensor(
                        stg[:], yp[:, :C], recipT[:, h, gt : gt + 1], prev,
                        ALU.mult, ALU.add,
                    )
                    nc.sync.dma_start(out_d.ap()[gt * 128 : (gt + 1) * 128, :], stg[:])
                else:
                    nc.vector.scalar_tensor_tensor(
                        dst[:, gt, :], yp[:, :C], recipT[:, h, gt : gt + 1], prev,
                        ALU.mult, ALU.add,
                    )

            # ---------------- attention ----------------
            def emit_attention(h, nci, jobs):
                n0 = nci * CHUNK
                ot = o_ps.tile([65, CHUNK], FP32, tag="ot")

                def emit_av(mp, pt):
                    for mt in (mp, mp + 1):
                        for c in range(2):
                            nc.tensor.matmul(
                                ot[:, c * 512 : c * 512 + 512],
                                v_sb[:, mt, h * VP : h * VP + 65],
                                pt[:, mt % 2, c * 512 : c * 512 + 512],
                                start=(mt == 0),
                                stop=(mt == MT - 1),
                            )

                prev = None
                for mp in range(0, MT, 2):
                    for _ in range(2):
                        if jobs:
                            jobs.pop(0)()
                    sp_e = s_ps.tile([128, 1024], FP32, tag="s")
                    sp_o = s_ps.tile([128, 1024], FP32, tag="s")
                    # interleave parities so the two PE row groups overlap
                    for c in range(2):
                        for par, sp in ((0, sp_e), (1, sp_o)):
                            rb = 64 * par
                            mt = mp + par
                            nc.tensor.matmul(
                                sp[:, c * 512 : c * 512 + 512],
                                kTd[rb : rb + 64, h, mt * 128 : (mt + 1) * 128],
                                qTd[rb : rb + 64, h, n0 + c * 512 : n0 + c * 512 + 512],
                                start=True,
                                stop=True,
                            )
                    if prev is not None:
                        emit_av(*prev)
                    pt = pt_pool.tile([128, 2, CHUNK], FP16, tag="pt")
                    nc.scalar.activation(
                        pt[:, 0, :], sp_e[:], AF.Exp, bias=bconst[:], scale=SCALE
                    )
                    nc.scalar.activation(
                        pt[:, 1, :], sp_o[:], AF.Exp, bias=bconst[:], scale=SCALE
                    )
                    if jobs:
                        jobs.pop(0)()
                    prev = (mp, pt)
                for job in jobs:
                    job()
                emit_av(*prev)
                nc.vector.tensor_copy(oT[:, h, n0 : n0 + CHUNK], ot[:])
                nc.sync.dma_start(
                    dstage[0:1, h, n0 : n0 + CHUNK], oT[64:65, h, n0 : n0 + CHUNK]
                )

            # ---------------- schedule ----------------
            for s0 in (0, 512):
                emit_qk_chunk(0, 0, s0)
                emit_qk_chunk(0, 1, s0)
            for mt in range(4):
                emit_v_tile(mt)
            J = lambda f, *a: (lambda: f(*a))
            jobs0 = [
                J(emit_xgrp, 8), J(emit_v_tile, 4), J(emit_v_tile, 5),
                J(emit_xgrp, 10), J(emit_v_tile, 6), J(emit_v_tile, 7),
                J(emit_qk_chunk, 0, 1, 1024), J(emit_xgrp, 12),
                J(emit_v_tile, 8), J(emit_v_tile, 9), J(emit_xgrp, 14),
                J(emit_v_tile, 10), J(emit_v_tile, 11),
                J(emit_qk_chunk, 0, 1, 1536), J(emit_v_tile, 12),
                J(emit_v_tile, 13), J(emit_v_tile, 14), J(emit_v_tile, 15),
                J(emit_qk_chunk, 0, 0, 1024), J(emit_qk_chunk, 0, 0, 1536),
            ]
            jobs1 = [J(emit_wproj, 0), J(emit_wproj, 1), J(emit_wproj, 2),
                     J(emit_bias)]
            for h in range(H):
                nxt = qk_jobs(h + 1) if h + 1 <= 2 else []
                projh = []
                if h >= 1:
                    projh = [lambda hh=h - 1: emit_recipT(hh)] + [
                        (lambda hh=h - 1, g=gt: emit_proj(hh, g)) for gt in range(MT)
                    ]
                if h == 0:
                    emit_attention(h, 0, jobs0)
                    emit_attention(h, 1, jobs1 + nxt)
                else:
                    emit_attention(h, 0, nxt[:4] + projh[:12])
                    emit_attention(h, 1, nxt[4:] + projh[12:])
            emit_recipT(H - 1)
            for gt in range(MT):
                emit_proj(H - 1, gt)

    nc.compile()
    return nc


_cache = {}


def _get_program(N: int):
    if N not in _cache:
        _cache[N] = build_program(N)
    return _cache[N]


def kernel(x, w_qkv, w_proj, b_proj):
    x = np.ascontiguousarray(np.asarray(x, dtype=np.float32))
    w_qkv = np.ascontiguousarray(np.asarray(w_qkv, dtype=np.float32))
    w_proj = np.ascontiguousarray(np.asarray(w_proj, dtype=np.float32))
    b_proj = np.ascontiguousarray(np.asarray(b_proj, dtype=np.float32))
    Bx, N, Cx = x.shape
    assert Bx == B and Cx == C, (x.shape,)

    nc = _get_program(N)
    in_maps = [
        {"x": x[b], "w_qkv": w_qkv, "w_proj": w_proj, "b_proj": b_proj}
        for b in range(B)
    ]
    res = run_bass_kernel_spmd(nc, in_maps, core_ids=list(range(B)))
    return np.stack([res.results[b]["out"] for b in range(B)], axis=0)


# revision 15
# speedup vs baseline: 1.0599x; 1.0321x over previous
"""Multi-head attention (B=8, N=2048, C=320, H=5, D=64) on 8 Trainium2 cores.

Sharding: data-parallel over batch — core b computes attention for x[b].
Weights replicated, no collectives.

Per-core engine plan (v2):
  - Scores S^T[m,n] in fp16 with PARITY ROW-PAIRING: q^T/k^T are stored
    per-head with the 64 d-features duplicated on partitions 64..127, so
    even-mt and odd-mt score matmuls use different PE row groups and run
    concurrently (~2x effective scores throughput).
  - exp is split between the scalar engine (true Exp -> fp8, scaled by 1/4
    via bias=-ln4 for fp8 range) and the vector engine (Schraudolph
    piecewise-linear 2^z: one tensor_scalar whose rounded uint8 output IS
    the fp8e4m3 encoding). The denominator is built from the same P values,
    so the PWL/quantization error largely normalizes out.
  - AV uses fp8 DoubleRow: stationary [V_h|1] m-block pairs [128,2,65]
    against P^T pairs [128,2,512] — half the AV matmuls; ones column gives
    the softmax denominator in row 64 of the output.
  - q/k/v projections: x^T kept as fp8 c-planes [128,2,N] + fp16 c-tail
    [64,N]; q/k pages produced via fp8-DoubleRow + fp16 tail directly into
    the duplicated layout (stationary w^T carries each head twice).
  - proj for head h runs inside head h+1's attention: denominators ride row
    64 of the oT eviction, are DMA'd (partition-shifting) to a base-0 row,
    PE-transposed per-gt into a [128,16] column, reciprocal'd once; one
    scalar_tensor_tensor per (gt,h) chains normalized projection partials
    with the bias seeding the chain.
  - gpsimd does the bulk x fp32->fp8/fp16 casts (it cannot touch PSUM).
"""

import numpy as np

import concourse.bacc as bacc
import concourse.tile as tile
from concourse import mybir
from concourse.bass_utils import run_bass_kernel_spmd
from concourse.masks import make_identity

FP32 = mybir.dt.float32
FP16 = mybir.dt.float16
FP8 = mybir.dt.float8e4
U8 = mybir.dt.uint8
AF = mybir.ActivationFunctionType
ALU = mybir.AluOpType
DRM = mybir.MatmulPerfMode.DoubleRow

B = 8
C = 320
H = 5
D = 64
SCALE = D ** -0.5
LN4 = 1.3862943611198906
# PWL fp8e4m3 exp bits: bits = 8*(log2e*SCALE*s - 2 + 7 + delta)
PWL_S1 = 1.4426950408889634 * SCALE * 8.0
PWL_S2 = 8.0 * (7.0 - 2.0 - 0.04295)
VP = 66  # v page stride per head (64 d + ones col + 1 pad)
VSTRIDE = 352  # per-mt stride in v_sb free dim (5*66=330 padded to 22*16)


def dve_exp_tile(h, nci, mt):
    """Which exp tiles go to the vector engine (PWL) vs scalar (true exp)."""
    return mt % 8 in (1, 3, 5)


def build_program(N: int):
    nc = bacc.Bacc("TRN2", target_bir_lowering=False, debug=False)

    x_d = nc.dram_tensor("x", [N, C], FP32, kind="ExternalInput")
    wqkv_d = nc.dram_tensor("w_qkv", [3 * C, C], FP32, kind="ExternalInput")
    wproj_d = nc.dram_tensor("w_proj", [C, C], FP32, kind="ExternalInput")
    bproj_d = nc.dram_tensor("b_proj", [C], FP32, kind="ExternalInput")
    out_d = nc.dram_tensor("out", [N, C], FP32, kind="ExternalOutput")

    MT = N // 128
    CHUNK = 1024
    assert N % CHUNK == 0 and MT % 2 == 0

    with tile.TileContext(nc) as tc:
        with (
            tc.tile_pool(name="per", bufs=1) as per,
            tc.tile_pool(name="ld", bufs=3) as ld,
            tc.tile_pool(name="s_ps", bufs=3, space="PSUM") as s_ps,
            tc.tile_pool(name="o_ps", bufs=1, space="PSUM") as o_ps,
            tc.tile_pool(name="pt", bufs=3) as pt_pool,
            tc.tile_pool(name="st", bufs=4) as st_pool,
        ):
            identity = per.tile([128, 128], FP32)
            make_identity(nc, identity[:])
            id16 = per.tile([128, 128], FP16)
            nc.vector.tensor_copy(id16[:], identity[:])

            # weights: transposed + head-duplicated layouts
            wplane = per.tile([128, 2, 2, 3, 128], FP16)  # [c, ci, qk, plane, col]
            wptl = per.tile([64, 2, 3, 128], FP16)  # c-tail 256..319
            wv = per.tile([128, 2, C], FP16)
            wv16 = per.tile([64, C], FP16)
            wpt = per.tile([64, H, C], FP16)
            bias_sb = per.tile([128, C], FP32)
            b_row = per.tile([1, C], FP32)
            ones1 = per.tile([1, 128], FP32)
            nc.gpsimd.memset(ones1[:], 1.0)
            bconst = per.tile([128, 1], FP32)
            nc.gpsimd.memset(bconst[:], -LN4)

            xTa = per.tile([128, 2, N], FP16)
            xT16 = per.tile([64, N], FP16)
            qTd = per.tile([128, H, N], FP16)
            kTd = per.tile([128, H, N], FP16)
            v_sb = per.tile([128, MT, VSTRIDE], FP16)
            oT = per.tile([65, H, N], FP16)  # rows 0..63 O^T/4, row 64 denom/4
            dstage = per.tile([1, H, N], FP16)
            recipT = per.tile([128, H, MT], FP32)
            acc_a = per.tile([128, MT, C], FP32)
            acc_b = per.tile([128, MT, C], FP32)

            v_heads = v_sb[:, :, 0 : H * VP].rearrange("p m (h e) -> p m h e", h=H)
            nc.gpsimd.memset(v_heads[:, :, :, D : D + 1], 1.0)

            # ---------------- weight prep ----------------
            # q/k rows of w_qkv: per qk, 3 blocks (128/128/64 rows)
            for qk in range(2):
                for wb in range(3):
                    r0 = qk * C + wb * 128
                    rp = min(128, C - wb * 128)
                    wnat = ld.tile([128, C], FP32, tag="wnat")
                    nc.sync.dma_start(wnat[:rp, :], wqkv_d.ap()[r0 : r0 + rp, :])
                    wnat_h = ld.tile([128, C], FP16, tag="wnat_h")
                    nc.vector.tensor_copy(wnat_h[:rp, :], wnat[:rp, :])
                    for ci in range(3):
                        c0, cp = ci * 128, (64 if ci == 2 else 128)
                        ps = s_ps.tile([128, 1024], FP32, tag="s")
                        psh = ps[:].bitcast(FP16)
                        nc.tensor.transpose(
                            psh[:cp, 0:rp], wnat_h[:rp, c0 : c0 + cp], id16[:rp, :rp]
                        )
                        if ci < 2:
                            nc.vector.tensor_copy(
                                wplane[:, ci, qk, wb, 0:rp], psh[:cp, 0:rp]
                            )
                        else:
                            nc.vector.tensor_copy(wptl[:, qk, wb, 0:rp], psh[:cp, 0:rp])
            # v rows of w_qkv
            for wb in range(3):
                r0 = 2 * C + wb * 128
                rp = min(128, C - wb * 128)
                wnat = ld.tile([128, C], FP32, tag="wnat")
                nc.sync.dma_start(wnat[:rp, :], wqkv_d.ap()[r0 : r0 + rp, :])
                wnat_h = ld.tile([128, C], FP16, tag="wnat_h")
                nc.vector.tensor_copy(wnat_h[:rp, :], wnat[:rp, :])
                for ci in range(3):
                    c0, cp = ci * 128, (64 if ci == 2 else 128)
                    ps = s_ps.tile([128, 1024], FP32, tag="s")
                    psh = ps[:].bitcast(FP16)
                    nc.tensor.transpose(
                        psh[:cp, 0:rp], wnat_h[:rp, c0 : c0 + cp], id16[:rp, :rp]
                    )
                    dst_cols = slice(wb * 128, wb * 128 + rp)
                    if ci < 2:
                        nc.vector.tensor_copy(wv[:, ci, dst_cols], psh[:cp, :rp])
                    else:
                        nc.vector.tensor_copy(wv16[:, dst_cols], psh[:cp, :rp])
            # w_proj -> wpt (emitted as jobs)
            def emit_wproj(wt):
                r0 = wt * 128
                rp = min(128, C - r0)
                wpnat = ld.tile([128, C], FP32, tag="wnat")
                nc.sync.dma_start(wpnat[:rp, :], wproj_d.ap()[r0 : r0 + rp, :])
                wpnat_h = ld.tile([128, C], FP16, tag="wnat_h")
                nc.vector.tensor_copy(wpnat_h[:rp, :], wpnat[:rp, :])
                for h in range(H):
                    ps = s_ps.tile([128, 1024], FP32, tag="s")
                    psh = ps[:].bitcast(FP16)
                    nc.tensor.transpose(
                        psh[:64, 0:rp],
                        wpnat_h[:rp, h * D : (h + 1) * D],
                        id16[:rp, :rp],
                    )
                    nc.vector.tensor_copy(wpt[:, h, r0 : r0 + rp], psh[:64, :rp])

            def emit_bias():
                nc.sync.dma_start(
                    b_row[:], bproj_d.ap().rearrange("(a c) -> a c", a=1)
                )
                ps = s_ps.tile([128, 1024], FP32, tag="s")
                nc.tensor.matmul(ps[:, :C], ones1[:], b_row[:], start=True, stop=True)
                nc.vector.tensor_copy(bias_sb[:], ps[:, :C])

            # ---------------- x load, cast, transpose ----------------
            x_re = x_d.ap().rearrange("(t p) c -> p t c", p=128)

            def emit_xgrp(g):
                gn = min(2, MT - g)
                xnat = ld.tile([128, 2, C], FP32, tag="xnat")
                nc.sync.dma_start(xnat[:, :gn, :], x_re[:, g : g + gn, :])
                xnath = ld.tile([128, 2, C], FP16, tag="xnath")
                eng = nc.vector if g <= 6 else nc.gpsimd
                eng.tensor_copy(xnath[:, :gn, :], xnat[:, :gn, :])
                for t in range(gn):
                    mt = g + t
                    ps = s_ps.tile([128, 1024], FP32, tag="s")
                    psh = ps[:].bitcast(FP16)
                    for ci in range(2):
                        nc.tensor.transpose(
                            psh[:, ci * 128 : ci * 128 + 128],
                            xnath[:, t, ci * 128 : ci * 128 + 128],
                            id16[:, :],
                        )
                    nc.tensor.transpose(
                        psh[:64, 256:384], xnath[:, t, 256:320], id16[:, :]
                    )
                    nc.vector.tensor_copy(
                        xTa[:, :, mt * 128 : (mt + 1) * 128],
                        psh[:, 0:256].rearrange("p (ci k) -> p ci k", ci=2),
                    )
                    nc.vector.tensor_copy(
                        xT16[:, mt * 128 : (mt + 1) * 128], psh[:64, 256:384]
                    )

            for g in (0, 2, 4, 6, 14):
                emit_xgrp(g)

            # ---------------- producers ----------------
            def emit_qk_chunk(j, qk, s0):
                # plane j covers heads (2j, 2j+1); j=2 covers head 4 only
                dst = qTd if qk == 0 else kTd
                nh = 2 if j < 2 else 1
                rows = 64 * nh
                h0 = 2 * j
                ps = s_ps.tile([128, 1024], FP32, tag="s")
                for ci in range(2):
                    nc.tensor.matmul(
                        ps[:rows, 0:512],
                        wplane[:, ci, qk, j, 0:rows],
                        xTa[:, ci, s0 : s0 + 512],
                        start=(ci == 0),
                        stop=False,
                    )
                nc.tensor.matmul(
                    ps[:rows, 0:512],
                    wptl[:, qk, j, 0:rows],
                    xT16[:, s0 : s0 + 512],
                    start=False,
                    stop=True,
                )
                # lane-aligned evictions into page halves, then DMA-duplicate
                nc.vector.tensor_copy(dst[0:64, h0, s0 : s0 + 512], ps[0:64, 0:512])
                nc.sync.dma_start(
                    dst[64:128, h0, s0 : s0 + 512], dst[0:64, h0, s0 : s0 + 512]
                )
                if nh == 2:
                    nc.vector.tensor_copy(
                        dst[64:128, h0 + 1, s0 : s0 + 512], ps[64:128, 0:512]
                    )
                    nc.sync.dma_start(
                        dst[0:64, h0 + 1, s0 : s0 + 512],
                        dst[64:128, h0 + 1, s0 : s0 + 512],
                    )

            def qk_jobs(j):
                return [
                    (lambda q=qk, s=s0: emit_qk_chunk(j, q, s))
                    for qk in range(2)
                    for s0 in range(0, N, 512)
                ]

            def emit_v_tile(mt):
                ps = s_ps.tile([128, 1024], FP32, tag="s")
                for ci in range(2):
                    nc.tensor.matmul(
                        ps[:, :C],
                        xTa[:, ci, mt * 128 : (mt + 1) * 128],
                        wv[:, ci, :],
                        start=(ci == 0),
                        stop=False,
                    )
                nc.tensor.matmul(
                    ps[:, :C],
                    xT16[:, mt * 128 : (mt + 1) * 128],
                    wv16[:, :],
                    start=False,
                    stop=True,
                )
                nc.vector.tensor_copy(
                    v_heads[:, mt, :, 0:D],
                    ps[:, :C].rearrange("p (h e) -> p h e", h=H),
                )

            # ---------------- per-head recipT / proj ----------------
            def emit_recipT(h, g0=0, g1=None):
                g1 = MT if g1 is None else g1
                rt0 = s_ps.tile([128, 1024], FP32, tag="s")
                rt = rt0[:, 0:32].bitcast(FP16)
                rtv = rt.rearrange("p (g two) -> p g two", two=2)
                for gt in range(g0, g1):
                    nc.tensor.transpose(
                        rtv[:, gt - g0, 0:1],
                        dstage[0:1, h, gt * 128 : (gt + 1) * 128],
                        id16[0:1, 0:1],
                    )
                nc.vector.tensor_copy(recipT[:, h, g0:g1], rtv[:, : g1 - g0, 0])
                nc.vector.reciprocal(recipT[:, h, g0:g1], recipT[:, h, g0:g1])

            def emit_proj(h, gt):
                yp = s_ps.tile([128, 1024], FP32, tag="s")
                nc.tensor.matmul(
                    yp[:, :C],
                    oT[0:64, h, gt * 128 : (gt + 1) * 128],
                    wpt[:, h, :],
                    start=True,
                    stop=True,
                )
                src = acc_a if h % 2 == 1 else acc_b
                dst = acc_a if h % 2 == 0 else acc_b
                prev = bias_sb[:] if h == 0 else src[:, gt, :]
                if h == H - 1:
                    stg = st_pool.tile([128, C], FP32, tag="ost")
                    nc.vector.scalar_tensor_tensor(
                        stg[:], yp[:, :C], recipT[:, h, gt : gt + 1], prev,
                        ALU.mult, ALU.add,
                    )
                    nc.sync.dma_start(out_d.ap()[gt * 128 : (gt + 1) * 128, :], stg[:])
                else:
                    nc.vector.scalar_tensor_tensor(
                        dst[:, gt, :], yp[:, :C], recipT[:, h, gt : gt + 1], prev,
                        ALU.mult, ALU.add,
                    )

            # ---------------- attention ----------------
            def emit_attention(h, nci, jobs):
                n0 = nci * CHUNK
                ot = o_ps.tile([65, CHUNK], FP32, tag="ot")

                def emit_av(mp, pt):
                    for mt in (mp, mp + 1):
                        for c in range(2):
                            nc.tensor.matmul(
                                ot[:, c * 512 : c * 512 + 512],
                                v_sb[:, mt, h * VP : h * VP + 65],
                                pt[:, mt % 2, c * 512 : c * 512 + 512],
                                start=(mt == 0),
                                stop=(mt == MT - 1),
                            )

                prev = None
                for mp in range(0, MT, 2):
                    if jobs:
                        jobs.pop(0)()
                    sp_e = s_ps.tile([128, 1024], FP32, tag="s")
                    sp_o = s_ps.tile([128, 1024], FP32, tag="s")
                    # interleave parities so the two PE row groups overlap
                    for c in range(2):
                        for par, sp in ((0, sp_e), (1, sp_o)):
                            rb = 64 * par
                            mt = mp + par
                            nc.tensor.matmul(
                                sp[:, c * 512 : c * 512 + 512],
                                kTd[rb : rb + 64, h, mt * 128 : (mt + 1) * 128],
                                qTd[rb : rb + 64, h, n0 + c * 512 : n0 + c * 512 + 512],
                                start=True,
                                stop=True,
                            )
                    if prev is not None:
                        emit_av(*prev)
                    pt = pt_pool.tile([128, 2, CHUNK], FP16, tag="pt")
                    nc.scalar.activation(
                        pt[:, 0, :], sp_e[:], AF.Exp, bias=bconst[:], scale=SCALE
                    )
                    nc.scalar.activation(
                        pt[:, 1, :], sp_o[:], AF.Exp, bias=bconst[:], scale=SCALE
                    )
                    if jobs:
                        jobs.pop(0)()
                    prev = (mp, pt)
                for job in jobs:
                    job()
                emit_av(*prev)
                for c in range(2):
                    nc.vector.tensor_copy(
                        oT[:, h, n0 + c * 512 : n0 + c * 512 + 512],
                        ot[:, c * 512 : c * 512 + 512],
                    )
                nc.sync.dma_start(
                    dstage[0:1, h, n0 : n0 + CHUNK], oT[64:65, h, n0 : n0 + CHUNK]
                )

            # ---------------- schedule ----------------
            for s0 in (0, 512):
                emit_qk_chunk(0, 0, s0)
                emit_qk_chunk(0, 1, s0)
            for mt in range(4):
                emit_v_tile(mt)
            J = lambda f, *a: (lambda: f(*a))
            jobs0 = [
                J(emit_xgrp, 8), J(emit_v_tile, 4), J(emit_v_tile, 5),
                J(emit_xgrp, 10), J(emit_v_tile, 6), J(emit_v_tile, 7),
                J(emit_xgrp, 12), J(emit_qk_chunk, 0, 1, 1024),
                J(emit_v_tile, 8), J(emit_v_tile, 9),
                J(emit_v_tile, 10), J(emit_v_tile, 11),
                J(emit_qk_chunk, 0, 1, 1536), J(emit_v_tile, 12),
                J(emit_v_tile, 13), J(emit_v_tile, 14), J(emit_v_tile, 15),
                J(emit_qk_chunk, 0, 0, 1024), J(emit_qk_chunk, 0, 0, 1536),
            ]
            jobs1 = [J(emit_wproj, 0), J(emit_wproj, 1), J(emit_wproj, 2),
                     J(emit_bias)]
            nop = lambda: None
            for h in range(H):
                nxt = qk_jobs(h + 1) if h + 1 <= 2 else []
                projh = []
                if h >= 1:
                    pad = [] if nxt else [nop, nop]
                    projh = pad + [lambda hh=h - 1: emit_recipT(hh)] + [
                        (lambda hh=h - 1, g=gt: emit_proj(hh, g)) for gt in range(MT)
                    ]
                if h == 0:
                    emit_attention(h, 0, jobs0)
                    emit_attention(h, 1, jobs1 + nxt)
                elif h < H - 1:
                    emit_attention(h, 0, nxt[:4] + projh[:12])
                    emit_attention(h, 1, nxt[4:] + projh[12:])
                else:
                    emit_attention(h, 0, projh[:14])
                    tail_jobs = projh[14:] + [
                        J(emit_recipT, h, 0, 8)
                    ] + [J(emit_proj, h, g) for g in range(8)]
                    emit_attention(h, 1, tail_jobs)
            emit_recipT(H - 1, 8, MT)
            for gt in range(8, MT):
                emit_proj(H - 1, gt)

    nc.compile()
    return nc


_cache = {}


def _get_program(N: int):
    if N not in _cache:
        _cache[N] = build_program(N)
    return _cache[N]


def kernel(x, w_qkv, w_proj, b_proj):
    x = np.ascontiguousarray(np.asarray(x, dtype=np.float32))
    w_qkv = np.ascontiguousarray(np.asarray(w_qkv, dtype=np.float32))
    w_proj = np.ascontiguousarray(np.asarray(w_proj, dtype=np.float32))
    b_proj = np.ascontiguousarray(np.asarray(b_proj, dtype=np.float32))
    Bx, N, Cx = x.shape
    assert Bx == B and Cx == C, (x.shape,)

    nc = _get_program(N)
    in_maps = [
        {"x": x[b], "w_qkv": w_qkv, "w_proj": w_proj, "b_proj": b_proj}
        for b in range(B)
    ]
    res = run_bass_kernel_spmd(nc, in_maps, core_ids=list(range(B)))
    return np.stack([res.results[b]["out"] for b in range(B)], axis=0)
